# revision 9
# baseline (speedup 1.0000x reference)
"""Cross-attention kernel for 8 trn2 NeuronCores.

Reference computation (per batch b of 16):
  q = Wq @ x, k = Wk @ y, v = Wv @ y          (1x1 convs as channel matmuls)
  q,k l2-normalized over the SPATIAL axis (per (h,d) row)
  sim = 10 * q^T k per head; attn = softmax_j(sim); o = attn @ v^T
  out = Wo @ o + b

Sharding: data-parallel over batch, 2 batches per core, weights replicated.

v2 design (ACT-bound plan; TimelineSim cost model):
  - exp runs on ACT only: 64 tiles x [128,1024] = 66.4us/core busy floor.
    Everything else is shaped to hide under it.
  - S_T (q^T k, contraction d=64) in fp8e4m3 with MatmulPerfMode.DoubleRow
    (0.5 cycles/row): q,k are separately l2-normalized (per hd-row scale via
    bn_stats + Quake rsqrt), quantized PSUM->SBUF fp8 by one tensor_scalar,
    then DMA-reshuffled (via DRAM scratch) into the DoubleRow pair layout
    [32p, 2pair, n] per head (d = p + 32*pair).
  - softmax denominator: ones-column in v^T (row 64 of PV out), normalized by
    gpsimd partition_broadcast of the PSUM denominator row + one DVE
    tensor_tensor divide per head (no DMA round trips).
  - PSUM (16KB/partition): st [128,1024]f32 bufs=2 (8K) + acc [128,512]f32
    bufs=2 (4K, proj/zproj halves) + op [65,1024]f32 bufs=1 (4K, PV accum).
    S_T writes 2KB-aligned [64,512] blocks (own start/stop groups).
  - PE emission inside a head is one-ahead: S_T(jc+1) is emitted before
    PV(jc) so ACT streams exps back to back.
  - PSUM->SBUF raw q/k copies and v^T copies run on gpsimd (Pool) to keep
    DVE under the ACT roofline.
"""

import sys

import numpy as np

if "/opt/trn_rl_repo" not in sys.path:
    sys.path.insert(0, "/opt/trn_rl_repo")

NB = 2        # batches per core
C = 256       # channels
N = 1024      # spatial (32*32)
HEADS = 4
DH = 64
HID = 256
NCORES = 8
MAGIC = 0x5F3759DF  # Quake fast inverse-sqrt seed

_CACHE = {}


def _quake_rsqrt(nc, pool, p_ap, out_ap, final_scale):
    """out = rsqrt(p) * final_scale for [128,1] fp32 APs, DVE-only.

    Quake seed + 2 Newton iterations (rel err ~1e-7), no ACT table needed.
    """
    from concourse import mybir

    i32 = mybir.dt.int32
    alu = mybir.AluOpType
    t = pool.tile([128, 1], mybir.dt.float32, tag="qk_rs_t", bufs=4)
    r = pool.tile([128, 1], mybir.dt.float32, tag="qk_rs_r", bufs=4)
    a = pool.tile([128, 1], mybir.dt.float32, tag="qk_rs_a", bufs=4)
    # seed: r0 = bitcast(MAGIC - (bitcast_i32(p) >> 1))
    nc.vector.tensor_scalar(t.bitcast(i32), p_ap.bitcast(i32), 1, None,
                            alu.logical_shift_right)
    nc.vector.tensor_scalar(r.bitcast(i32), t.bitcast(i32), -1, MAGIC,
                            alu.mult, alu.add)
    # Newton 1: r = r * (1.5 - 0.5 * p * r^2)
    nc.vector.scalar_tensor_tensor(a[:], r[:], r[:, 0:1], p_ap,
                                   alu.mult, alu.mult)
    nc.vector.tensor_scalar(a[:], a[:], -0.5, 1.5, alu.mult, alu.add)
    nc.vector.tensor_scalar(t[:], a[:], r[:, 0:1], None, alu.mult)
    # Newton 2 (fold final_scale into the last multiply)
    nc.vector.scalar_tensor_tensor(a[:], t[:], t[:, 0:1], p_ap,
                                   alu.mult, alu.mult)
    nc.vector.tensor_scalar(a[:], a[:], -0.5, 1.5, alu.mult, alu.add)
    nc.vector.tensor_scalar(out_ap, a[:], t[:, 0:1], final_scale,
                            alu.mult, alu.mult)


def _build_nc():
    from contextlib import ExitStack

    import concourse.tile as tile
    from concourse import bacc, mybir

    f32 = mybir.dt.float32
    f16 = mybir.dt.float16
    f8 = mybir.dt.float8e4
    alu = mybir.AluOpType
    EXP = mybir.ActivationFunctionType.Exp
    DR = mybir.MatmulPerfMode.DoubleRow

    nc = bacc.Bacc("TRN2", target_bir_lowering=False)

    xin = nc.dram_tensor("x", [NB, C, N], f16, kind="ExternalInput")
    yin = nc.dram_tensor("y", [NB, C, N], f16, kind="ExternalInput")
    wq = nc.dram_tensor("wq_t", [C, HID], f16, kind="ExternalInput")
    wk = nc.dram_tensor("wk_t", [C, HID], f16, kind="ExternalInput")
    wv = nc.dram_tensor("wv_t", [C, HID], f16, kind="ExternalInput")
    wo = nc.dram_tensor("wo_t", [HID, C], f16, kind="ExternalInput")
    bo = nc.dram_tensor("b_out", [2, 128, 1], f32, kind="ExternalInput")
    out = nc.dram_tensor("out", [NB, C, N], f32, kind="ExternalOutput")
    # DRAM scratch for the fp8 DoubleRow pair-layout reshuffle
    q8d = nc.dram_tensor("q8_scratch", [NB, 2, 128, N], f8, kind="Internal")
    k8d = nc.dram_tensor("k8_scratch", [NB, 2, 128, N], f8, kind="Internal")

    with tile.TileContext(nc) as tc, ExitStack() as ctx:
        consts = ctx.enter_context(tc.tile_pool(name="consts", bufs=1))
        big = ctx.enter_context(tc.tile_pool(name="big", bufs=2))
        sm = ctx.enter_context(tc.tile_pool(name="sm", bufs=4))
        ps = ctx.enter_context(tc.tile_pool(name="ps", bufs=2, space="PSUM"))

        # ---- weight + input loads (k/q path first: critical) ----------
        wq_sb = consts.tile([128, 2, HID], f16, tag="wq")
        wk_sb = consts.tile([128, 2, HID], f16, tag="wk")
        wv_sb = consts.tile([128, 2, HID], f16, tag="wv")
        wo_sb = consts.tile([128, 2, C], f16, tag="wo")
        b_sb = consts.tile([128, 2, 1], f32, tag="bo")
        # warm the ACT exp table while input DMAs are in flight
        warm = sm.tile([128, 1], f32, tag="warm", bufs=1)
        nc.vector.memset(warm[:], 0.0)
        nc.scalar.activation(out=warm[:], in_=warm[:], func=EXP, scale=1.0)
        xts, yts = [], []
        for nb in range(NB):
            xt = big.tile([128, 2, N], f16, tag="xt", bufs=2)
            yt = big.tile([128, 2, N], f16, tag="yt", bufs=2)
            xts.append(xt)
            yts.append(yt)
        nc.sync.dma_start(out=wq_sb[:], in_=wq.rearrange("(kc p) n -> p kc n", p=128))
        for kc in range(2):
            nc.sync.dma_start(out=xts[0][:, kc, :], in_=xin[0, kc * 128:(kc + 1) * 128, :])
        nc.sync.dma_start(out=wk_sb[:], in_=wk.rearrange("(kc p) n -> p kc n", p=128))
        for kc in range(2):
            nc.sync.dma_start(out=yts[0][:, kc, :], in_=yin[0, kc * 128:(kc + 1) * 128, :])
        nc.sync.dma_start(out=wv_sb[:], in_=wv.rearrange("(kc p) n -> p kc n", p=128))
        nc.sync.dma_start(out=wo_sb[:], in_=wo.rearrange("(kc p) n -> p kc n", p=128))
        nc.sync.dma_start(out=b_sb[:], in_=bo.rearrange("kc p n -> p kc n"))
        nc.sync.dma_start(out=yts[1][:], in_=yin[1].rearrange("(kc p) n -> p kc n", p=128))
        nc.sync.dma_start(out=xts[1][:], in_=xin[1].rearrange("(kc p) n -> p kc n", p=128))

        # ---------------------------------------------------------------
        # proj_qk: q,k channel matmuls for one 128-row hd chunk (2 heads),
        # separate l2 scales, fp8 quantize, DoubleRow reshuffle via DRAM.
        # q8r/k8r: [64, 2, N] fp8; partitions [ha*32,(ha+1)*32) = head
        # 2*mc+ha, pair dim = d split (d = p + 32*pair).
        # Emitted in 4 stages so PSUM 'acc' ring waits never head-block the
        # PE stream of a concurrently-running attention head.
        def _proj_one(nb, mc, w_sb, src, raw_tag, t8_tag, dram, t8r, state):
            def mm():
                state["ps"] = []
                for ih in range(2):
                    pp = ps.tile([128, 512], f32, tag="acc", bufs=2, name="pp")
                    for kc in range(2):
                        nc.tensor.matmul(
                            pp[:],
                            w_sb[:, kc, mc * 128:(mc + 1) * 128],
                            src[:, kc, ih * 512:(ih + 1) * 512],
                            start=(kc == 0), stop=(kc == 1))
                    state["ps"].append(pp)

            def quant():
                pps = state["ps"]
                raw = big.tile([128, N], f32, tag=raw_tag, bufs=2, name=raw_tag)
                st = sm.tile([128, 2, 6], f32, tag="st" + raw_tag, bufs=4, name="st" + raw_tag)
                for ih in range(2):
                    nc.vector.bn_stats(out=st[:, ih, :], in_=pps[ih][:])
                    nc.gpsimd.tensor_copy(raw[:, ih * 512:(ih + 1) * 512],
                                          pps[ih][:])
                mv = sm.tile([128, 2], f32, tag="mv" + raw_tag, bufs=4, name="mv" + raw_tag)
                nc.vector.bn_aggr(out=mv[:], in_=st[:])
                # sumsq/N = mean^2 + var; scale = rsqrt(u)/sqrt(N)
                u = sm.tile([128, 1], f32, tag="u" + raw_tag, bufs=4, name="u" + raw_tag)
                nc.vector.scalar_tensor_tensor(u[:], mv[:, 0:1], mv[:, 0:1],
                                               mv[:, 1:2], alu.mult, alu.add)
                sc = sm.tile([128, 1], f32, tag="s" + raw_tag, bufs=4, name="s" + raw_tag)
                _quake_rsqrt(nc, sm, u[:], sc[:], 1.0 / 32.0)
                t8 = big.tile([128, N], f8, tag=t8_tag, bufs=2, name=t8_tag)
                nc.vector.tensor_scalar(t8[:], raw[:], sc[:, 0:1], None, alu.mult)
                nc.sync.dma_start(out=dram[nb, mc], in_=t8[:])
                for ha in range(2):
                    nc.sync.dma_start(
                        out=t8r[ha * 32:(ha + 1) * 32, :, :],
                        in_=dram[nb, mc, ha * 64:(ha + 1) * 64, :].rearrange(
                            "(i p) n -> p i n", i=2, p=32))

            return mm, quant

        def proj_qk_stages(nb, mc, q8r, k8r):
            stq, stk = {}, {}
            qmm, qquant = _proj_one(nb, mc, wq_sb, xts[nb], "qraw", "q8",
                                    q8d, q8r, stq)
            kmm, kquant = _proj_one(nb, mc, wk_sb, yts[nb], "kraw", "k8",
                                    k8d, k8r, stk)
            return [qmm, qquant, kmm, kquant]

        def proj_v(nb, jcs, vts_nb):
            for jc in jcs:
                vp = ps.tile([128, 512], f32, tag="acc", bufs=2)
                for kc in range(2):
                    nc.tensor.matmul(
                        vp[:, 0:HID],
                        yts[nb][:, kc, jc * 128:(jc + 1) * 128],
                        wv_sb[:, kc, :],
                        start=(kc == 0), stop=(kc == 1))
                vt = big.tile([128, 4, 65], f16, tag="vt", bufs=16)
                nc.gpsimd.tensor_copy(vt[:, :, 0:64],
                                      vp[:, 0:HID].rearrange("p (h d) -> p h d", h=4))
                nc.gpsimd.memset(vt[:, :, 64:65], 1.0)
                vts_nb.append(vt)

        # one attention head, emitted as a generator of "steps" so PE filler
        # work can be interleaved between jc iterations without delaying the
        # ACT-critical S_T / exp chain.
        def attn_head(nb, h, q8r2, k8r2, vts_nb, o_sb):
            mc, ha = h // 2, h % 2
            q8r, k8r = q8r2[mc], k8r2[mc]
            hp, hr = mc, 64 * ha

            def st_mm(jc):
                st = ps.tile([128, N], f32, tag="st", bufs=2)
                for j64 in range(2):
                    for ih in range(2):
                        nc.tensor.matmul(
                            st[j64 * 64:(j64 + 1) * 64, ih * 512:(ih + 1) * 512],
                            k8r[ha * 32:(ha + 1) * 32, :,
                                jc * 128 + j64 * 64: jc * 128 + (j64 + 1) * 64],
                            q8r[ha * 32:(ha + 1) * 32, :, ih * 512:(ih + 1) * 512],
                            start=True, stop=True, perf_mode=DR)
                return st

            op = ps.tile([65, N], f32, tag="op", bufs=1)
            sts = [st_mm(0)]
            for jc in range(8):
                et = big.tile([128, N], f16, tag="et", bufs=6)
                nc.scalar.activation(out=et[:], in_=sts[jc][:], func=EXP, scale=10.0)
                if jc < 7:
                    sts.append(st_mm(jc + 1))
                vt = vts_nb[jc]
                for ih in range(2):
                    nc.tensor.matmul(
                        op[:, ih * 512:(ih + 1) * 512],
                        vt[:, h, :],
                        et[:, ih * 512:(ih + 1) * 512],
                        start=(jc == 0), stop=(jc == 7))
                yield jc
            # softmax normalize: broadcast denominator row, one divide
            db = big.tile([64, N], f32, tag="db", bufs=2)
            nc.gpsimd.partition_broadcast(db[:], op[64:65, :])
            nc.vector.tensor_tensor(o_sb[hr:hr + 64, hp, :], op[0:64, :], db[:],
                                    alu.divide)
            yield -1

        def run_head(gen, fillers=()):
            """Drive a head generator, emitting one filler callable per step."""
            fi = iter(fillers)
            for _ in gen:
                f = next(fi, None)
                if f is not None:
                    f()
            for f in fi:
                f()

        def zproj(nb, o_sb, ihs=(0, 1)):
            for mc in range(2):
                for ih in ihs:
                    zp = ps.tile([128, 512], f32, tag="acc", bufs=2)
                    for kc in range(2):
                        nc.tensor.matmul(
                            zp[:],
                            wo_sb[:, kc, mc * 128:(mc + 1) * 128],
                            o_sb[:, kc, ih * 512:(ih + 1) * 512],
                            start=(kc == 0), stop=(kc == 1))
                    zs = big.tile([128, 512], f32, tag="zs", bufs=4)
                    nc.vector.tensor_scalar(zs[:], zp[:], b_sb[:, mc, 0:1], None,
                                            alu.add)
                    nc.sync.dma_start(
                        out=out[nb, mc * 128:(mc + 1) * 128, ih * 512:(ih + 1) * 512],
                        in_=zs[:])

        def alloc_qk():
            q8r2 = [big.tile([64, 2, N], f8, tag="q8r", bufs=4, name=f"q8r{i}")
                    for i in range(2)]
            k8r2 = [big.tile([64, 2, N], f8, tag="k8r", bufs=4, name=f"k8r{i}")
                    for i in range(2)]
            return q8r2, k8r2

        def alloc_o():
            return big.tile([128, 2, N], f16, tag="osb", bufs=2, name="osb")

        # ---- schedule -------------------------------------------------
        q8r_0, k8r_0 = alloc_qk()
        q8r_1, k8r_1 = alloc_qk()
        o0 = alloc_o()
        o1 = alloc_o()
        vts0, vts1 = [], []
        # batch-0 mc0 q/k (startup critical), then first v tiles
        for s in proj_qk_stages(0, 0, q8r_0[0], k8r_0[0]):
            s()
        proj_v(0, [0, 1, 2], vts0)
        qk01 = proj_qk_stages(0, 1, q8r_0[1], k8r_0[1])
        qk10 = proj_qk_stages(1, 0, q8r_1[0], k8r_1[0])
        qk11 = proj_qk_stages(1, 1, q8r_1[1], k8r_1[1])
        run_head(attn_head(0, 0, q8r_0, k8r_0, vts0, o0),
                 [lambda: proj_v(0, [3, 4], vts0),
                  lambda: proj_v(0, [5, 6], vts0),
                  lambda: proj_v(0, [7], vts0),
                  qk01[0], qk01[1], qk01[2], qk01[3]])
        run_head(attn_head(0, 1, q8r_0, k8r_0, vts0, o0),
                 [qk10[0], qk10[1], qk10[2], qk10[3]])
        run_head(attn_head(0, 2, q8r_0, k8r_0, vts0, o0),
                 [lambda: proj_v(1, [0, 1], vts1),
                  lambda: proj_v(1, [2, 3], vts1),
                  qk11[0], qk11[1], qk11[2], qk11[3]])
        run_head(attn_head(0, 3, q8r_0, k8r_0, vts0, o0),
                 [lambda: proj_v(1, [4, 5], vts1),
                  lambda: proj_v(1, [6, 7], vts1)])
        # batch 1
        run_head(attn_head(1, 0, q8r_1, k8r_1, vts1, o1),
                 [lambda: zproj(0, o0, (0,)),
                  lambda: zproj(0, o0, (1,))])
        run_head(attn_head(1, 1, q8r_1, k8r_1, vts1, o1))
        run_head(attn_head(1, 2, q8r_1, k8r_1, vts1, o1))
        run_head(attn_head(1, 3, q8r_1, k8r_1, vts1, o1))
        zproj(1, o1)

    nc.finalize()
    return nc


def _get_nc():
    if "nc" not in _CACHE:
        _CACHE["nc"] = _build_nc()
    return _CACHE["nc"]


def kernel(x, y, w_qkv, w_out, b_out):
    from concourse.bass_utils import run_bass_kernel_spmd

    nc = _get_nc()

    x = np.asarray(x, dtype=np.float32).reshape(16, C, N).astype(np.float16)
    y = np.asarray(y, dtype=np.float32).reshape(16, C, N).astype(np.float16)
    w_qkv = np.asarray(w_qkv, dtype=np.float32)
    wq_t = np.ascontiguousarray(w_qkv[0:HID].T).astype(np.float16)
    wk_t = np.ascontiguousarray(w_qkv[HID:2 * HID].T).astype(np.float16)
    wv_t = np.ascontiguousarray(w_qkv[2 * HID:3 * HID].T).astype(np.float16)
    wo_t = np.ascontiguousarray(np.asarray(w_out, dtype=np.float32).T).astype(np.float16)
    bo = np.ascontiguousarray(
        np.asarray(b_out, dtype=np.float32).reshape(2, 128, 1))

    in_maps = []
    for c in range(NCORES):
        in_maps.append({
            "x": np.ascontiguousarray(x[c * NB:(c + 1) * NB]),
            "y": np.ascontiguousarray(y[c * NB:(c + 1) * NB]),
            "wq_t": wq_t, "wk_t": wk_t, "wv_t": wv_t, "wo_t": wo_t,
            "b_out": bo,
        })

    res = run_bass_kernel_spmd(nc, in_maps, list(range(NCORES)))
    full = np.concatenate([res.results[i]["out"] for i in range(NCORES)], axis=0)
    return full.reshape(16, C, 32, 32)


# revision 10
# speedup vs baseline: 1.0560x; 1.0560x over previous
"""Cross-attention kernel for 8 trn2 NeuronCores.

Reference computation (per batch b of 16):
  q = Wq @ x, k = Wk @ y, v = Wv @ y          (1x1 convs as channel matmuls)
  q,k l2-normalized over the SPATIAL axis (per (h,d) row)
  sim = 10 * q^T k per head; attn = softmax_j(sim); o = attn @ v^T
  out = Wo @ o + b

Sharding: data-parallel over batch, 2 batches per core, weights replicated.

v2 design (ACT-bound plan; TimelineSim cost model):
  - exp runs on ACT only: 64 tiles x [128,1024] = 66.4us/core busy floor.
    Everything else is shaped to hide under it.
  - S_T (q^T k, contraction d=64) in fp8e4m3 with MatmulPerfMode.DoubleRow
    (0.5 cycles/row): q,k are separately l2-normalized (per hd-row scale via
    bn_stats + Quake rsqrt), quantized PSUM->SBUF fp8 by one tensor_scalar,
    then DMA-reshuffled (via DRAM scratch) into the DoubleRow pair layout
    [32p, 2pair, n] per head (d = p + 32*pair).
  - softmax denominator: ones-column in v^T (row 64 of PV out), normalized by
    gpsimd partition_broadcast of the PSUM denominator row + one DVE
    tensor_tensor divide per head (no DMA round trips).
  - PSUM (16KB/partition): st [128,1024]f32 bufs=2 (8K) + acc [128,512]f32
    bufs=2 (4K, proj/zproj halves) + op [65,1024]f32 bufs=1 (4K, PV accum).
    S_T writes 2KB-aligned [64,512] blocks (own start/stop groups).
  - PE emission inside a head is one-ahead: S_T(jc+1) is emitted before
    PV(jc) so ACT streams exps back to back.
  - PSUM->SBUF raw q/k copies and v^T copies run on gpsimd (Pool) to keep
    DVE under the ACT roofline.
"""

import sys

import numpy as np

if "/opt/trn_rl_repo" not in sys.path:
    sys.path.insert(0, "/opt/trn_rl_repo")

NB = 2        # batches per core
C = 256       # channels
N = 1024      # spatial (32*32)
HEADS = 4
DH = 64
HID = 256
NCORES = 8
MAGIC = 0x5F3759DF  # Quake fast inverse-sqrt seed

_CACHE = {}


def _quake_rsqrt(nc, pool, p_ap, out_ap, final_scale):
    """out = rsqrt(p) * final_scale for [128,1] fp32 APs, DVE-only.

    Quake seed + 2 Newton iterations (rel err ~1e-7), no ACT table needed.
    """
    from concourse import mybir

    i32 = mybir.dt.int32
    alu = mybir.AluOpType
    t = pool.tile([128, 1], mybir.dt.float32, tag="qk_rs_t", bufs=4)
    r = pool.tile([128, 1], mybir.dt.float32, tag="qk_rs_r", bufs=4)
    a = pool.tile([128, 1], mybir.dt.float32, tag="qk_rs_a", bufs=4)
    # seed: r0 = bitcast(MAGIC - (bitcast_i32(p) >> 1))
    nc.vector.tensor_scalar(t.bitcast(i32), p_ap.bitcast(i32), 1, None,
                            alu.logical_shift_right)
    nc.vector.tensor_scalar(r.bitcast(i32), t.bitcast(i32), -1, MAGIC,
                            alu.mult, alu.add)
    # Newton 1: r = r * (1.5 - 0.5 * p * r^2)
    nc.vector.scalar_tensor_tensor(a[:], r[:], r[:, 0:1], p_ap,
                                   alu.mult, alu.mult)
    nc.vector.tensor_scalar(a[:], a[:], -0.5, 1.5, alu.mult, alu.add)
    nc.vector.tensor_scalar(t[:], a[:], r[:, 0:1], None, alu.mult)
    # Newton 2 (fold final_scale into the last multiply)
    nc.vector.scalar_tensor_tensor(a[:], t[:], t[:, 0:1], p_ap,
                                   alu.mult, alu.mult)
    nc.vector.tensor_scalar(a[:], a[:], -0.5, 1.5, alu.mult, alu.add)
    nc.vector.tensor_scalar(out_ap, a[:], t[:, 0:1], final_scale,
                            alu.mult, alu.mult)


def _build_nc():
    from contextlib import ExitStack

    import concourse.tile as tile
    from concourse import bacc, mybir

    f32 = mybir.dt.float32
    f16 = mybir.dt.float16
    f8 = mybir.dt.float8e4
    alu = mybir.AluOpType
    EXP = mybir.ActivationFunctionType.Exp
    DR = mybir.MatmulPerfMode.DoubleRow

    nc = bacc.Bacc("TRN2", target_bir_lowering=False)

    xin = nc.dram_tensor("x", [NB, C, N], f16, kind="ExternalInput")
    yin = nc.dram_tensor("y", [NB, C, N], f16, kind="ExternalInput")
    wq = nc.dram_tensor("wq_t", [C, HID], f16, kind="ExternalInput")
    wk = nc.dram_tensor("wk_t", [C, HID], f16, kind="ExternalInput")
    wv = nc.dram_tensor("wv_t", [C, HID], f16, kind="ExternalInput")
    wo = nc.dram_tensor("wo_t", [HID, C], f16, kind="ExternalInput")
    bo = nc.dram_tensor("b_out", [2, 128, 1], f32, kind="ExternalInput")
    out = nc.dram_tensor("out", [NB, C, N], f32, kind="ExternalOutput")
    # DRAM scratch for the fp8 DoubleRow pair-layout reshuffle
    q8d = nc.dram_tensor("q8_scratch", [NB, 2, 128, N], f8, kind="Internal")
    k8d = nc.dram_tensor("k8_scratch", [NB, 2, 128, N], f8, kind="Internal")

    with tile.TileContext(nc) as tc, ExitStack() as ctx:
        consts = ctx.enter_context(tc.tile_pool(name="consts", bufs=1))
        big = ctx.enter_context(tc.tile_pool(name="big", bufs=2))
        sm = ctx.enter_context(tc.tile_pool(name="sm", bufs=4))
        ps = ctx.enter_context(tc.tile_pool(name="ps", bufs=2, space="PSUM"))

        # ---- weight + input loads (k/q path first: critical) ----------
        wq_sb = consts.tile([128, 2, HID], f16, tag="wq")
        wk_sb = consts.tile([128, 2, HID], f16, tag="wk")
        wv_sb = consts.tile([128, 2, HID], f16, tag="wv")
        wo_sb = consts.tile([128, 2, C], f16, tag="wo")
        b_sb = consts.tile([128, 2, 1], f32, tag="bo")
        # warm the ACT exp table while input DMAs are in flight
        warm = sm.tile([128, 1], f32, tag="warm", bufs=1)
        nc.vector.memset(warm[:], 0.0)
        nc.scalar.activation(out=warm[:], in_=warm[:], func=EXP, scale=1.0)
        xts, yts = [], []
        for nb in range(NB):
            xt = big.tile([128, 2, N], f16, tag="xt", bufs=2)
            yt = big.tile([128, 2, N], f16, tag="yt", bufs=2)
            xts.append(xt)
            yts.append(yt)
        nc.sync.dma_start(out=wq_sb[:], in_=wq.rearrange("(kc p) n -> p kc n", p=128))
        for kc in range(2):
            nc.sync.dma_start(out=xts[0][:, kc, :], in_=xin[0, kc * 128:(kc + 1) * 128, :])
        nc.sync.dma_start(out=wk_sb[:], in_=wk.rearrange("(kc p) n -> p kc n", p=128))
        for kc in range(2):
            nc.sync.dma_start(out=yts[0][:, kc, :], in_=yin[0, kc * 128:(kc + 1) * 128, :])
        nc.sync.dma_start(out=wv_sb[:], in_=wv.rearrange("(kc p) n -> p kc n", p=128))
        nc.sync.dma_start(out=wo_sb[:], in_=wo.rearrange("(kc p) n -> p kc n", p=128))
        nc.sync.dma_start(out=b_sb[:], in_=bo.rearrange("kc p n -> p kc n"))
        nc.sync.dma_start(out=yts[1][:], in_=yin[1].rearrange("(kc p) n -> p kc n", p=128))
        nc.sync.dma_start(out=xts[1][:], in_=xin[1].rearrange("(kc p) n -> p kc n", p=128))

        # ---------------------------------------------------------------
        # proj_qk: q,k channel matmuls for one 128-row hd chunk (2 heads),
        # separate l2 scales, fp8 quantize, DoubleRow reshuffle via DRAM.
        # q8r/k8r: [64, 2, N] fp8; partitions [ha*32,(ha+1)*32) = head
        # 2*mc+ha, pair dim = d split (d = p + 32*pair).
        # Emitted in 4 stages so PSUM 'acc' ring waits never head-block the
        # PE stream of a concurrently-running attention head.
        def _proj_one(nb, mc, w_sb, src, raw_tag, t8_tag, dram, t8r, state):
            def mm():
                state["ps"] = []
                for ih in range(2):
                    pp = ps.tile([128, 512], f32, tag="acc", bufs=2, name="pp")
                    for kc in range(2):
                        nc.tensor.matmul(
                            pp[:],
                            w_sb[:, kc, mc * 128:(mc + 1) * 128],
                            src[:, kc, ih * 512:(ih + 1) * 512],
                            start=(kc == 0), stop=(kc == 1))
                    state["ps"].append(pp)

            def quant():
                pps = state["ps"]
                st = sm.tile([128, 2, 6], f32, tag="st" + raw_tag, bufs=4, name="st" + raw_tag)
                for ih in range(2):
                    nc.vector.bn_stats(out=st[:, ih, :], in_=pps[ih][:])
                mv = sm.tile([128, 2], f32, tag="mv" + raw_tag, bufs=4, name="mv" + raw_tag)
                nc.vector.bn_aggr(out=mv[:], in_=st[:])
                # sumsq/N = mean^2 + var; scale = rsqrt(u)/sqrt(N)
                u = sm.tile([128, 1], f32, tag="u" + raw_tag, bufs=4, name="u" + raw_tag)
                nc.vector.scalar_tensor_tensor(u[:], mv[:, 0:1], mv[:, 0:1],
                                               mv[:, 1:2], alu.mult, alu.add)
                sc = sm.tile([128, 1], f32, tag="s" + raw_tag, bufs=4, name="s" + raw_tag)
                _quake_rsqrt(nc, sm, u[:], sc[:], 1.0 / 32.0)
                t8 = big.tile([128, N], f8, tag=t8_tag, bufs=2, name=t8_tag)
                for ih in range(2):
                    nc.vector.tensor_scalar(t8[:, ih * 512:(ih + 1) * 512],
                                            pps[ih][:], sc[:, 0:1], None, alu.mult)
                nc.sync.dma_start(out=dram[nb, mc], in_=t8[:])
                for ha in range(2):
                    nc.sync.dma_start(
                        out=t8r[ha * 32:(ha + 1) * 32, :, :],
                        in_=dram[nb, mc, ha * 64:(ha + 1) * 64, :].rearrange(
                            "(i p) n -> p i n", i=2, p=32))

            return mm, quant

        def proj_qk_stages(nb, mc, q8r, k8r):
            stq, stk = {}, {}
            qmm, qquant = _proj_one(nb, mc, wq_sb, xts[nb], "qraw", "q8",
                                    q8d, q8r, stq)
            kmm, kquant = _proj_one(nb, mc, wk_sb, yts[nb], "kraw", "k8",
                                    k8d, k8r, stk)
            return [qmm, qquant, kmm, kquant]

        def proj_v(nb, jcs, vts_nb):
            for jc in jcs:
                vp = ps.tile([128, 512], f32, tag="acc", bufs=2)
                for kc in range(2):
                    nc.tensor.matmul(
                        vp[:, 0:HID],
                        yts[nb][:, kc, jc * 128:(jc + 1) * 128],
                        wv_sb[:, kc, :],
                        start=(kc == 0), stop=(kc == 1))
                vt = big.tile([128, 4, 128], f16, tag="vt", bufs=16)
                nc.vector.tensor_copy(vt[:, :, 0:64],
                                      vp[:, 0:HID].rearrange("p (h d) -> p h d", h=4))
                nc.gpsimd.memset(vt[:, :, 64:128], 1.0)
                vts_nb.append(vt)

        # one attention head, emitted as a generator of "steps" so PE filler
        # work can be interleaved between jc iterations without delaying the
        # ACT-critical S_T / exp chain.
        def attn_head(nb, h, q8r2, k8r2, vts_nb, o_sb):
            mc, ha = h // 2, h % 2
            q8r, k8r = q8r2[mc], k8r2[mc]
            hp, hr = mc, 64 * ha

            def st_mm(jc):
                st = ps.tile([128, N], f32, tag="st", bufs=2)
                for j64 in range(2):
                    for ih in range(2):
                        nc.tensor.matmul(
                            st[j64 * 64:(j64 + 1) * 64, ih * 512:(ih + 1) * 512],
                            k8r[ha * 32:(ha + 1) * 32, :,
                                jc * 128 + j64 * 64: jc * 128 + (j64 + 1) * 64],
                            q8r[ha * 32:(ha + 1) * 32, :, ih * 512:(ih + 1) * 512],
                            start=True, stop=True, perf_mode=DR)
                return st

            op = ps.tile([128, N], f32, tag="op", bufs=1)
            sts = [st_mm(0)]
            for jc in range(8):
                et = big.tile([128, N], f16, tag="et", bufs=6)
                nc.scalar.activation(out=et[:], in_=sts[jc][:], func=EXP, scale=10.0)
                if jc < 7:
                    sts.append(st_mm(jc + 1))
                vt = vts_nb[jc]
                for ih in range(2):
                    nc.tensor.matmul(
                        op[:, ih * 512:(ih + 1) * 512],
                        vt[:, h, :],
                        et[:, ih * 512:(ih + 1) * 512],
                        start=(jc == 0), stop=(jc == 7))
                yield jc
            # softmax normalize: denominator is pre-broadcast on rows 64:128
            # (64-wide ones block in vt), one PSUM-sourced divide
            nc.vector.tensor_tensor(o_sb[hr:hr + 64, hp, :], op[0:64, :],
                                    op[64:128, :], alu.divide)
            yield -1

        def run_head(gen, fillers=()):
            """Drive a head generator, emitting one filler callable per step."""
            fi = iter(fillers)
            for _ in gen:
                f = next(fi, None)
                if f is not None:
                    f()
            for f in fi:
                f()

        def zproj(nb, o_sb, ihs=(0, 1)):
            for mc in range(2):
                for ih in ihs:
                    zp = ps.tile([128, 512], f32, tag="acc", bufs=2)
                    for kc in range(2):
                        nc.tensor.matmul(
                            zp[:],
                            wo_sb[:, kc, mc * 128:(mc + 1) * 128],
                            o_sb[:, kc, ih * 512:(ih + 1) * 512],
                            start=(kc == 0), stop=(kc == 1))
                    zs = big.tile([128, 512], f32, tag="zs", bufs=4)
                    nc.vector.tensor_scalar(zs[:], zp[:], b_sb[:, mc, 0:1], None,
                                            alu.add)
                    nc.sync.dma_start(
                        out=out[nb, mc * 128:(mc + 1) * 128, ih * 512:(ih + 1) * 512],
                        in_=zs[:])

        def alloc_qk():
            q8r2 = [big.tile([64, 2, N], f8, tag="q8r", bufs=4, name=f"q8r{i}")
                    for i in range(2)]
            k8r2 = [big.tile([64, 2, N], f8, tag="k8r", bufs=4, name=f"k8r{i}")
                    for i in range(2)]
            return q8r2, k8r2

        def alloc_o():
            return big.tile([128, 2, N], f16, tag="osb", bufs=2, name="osb")

        # ---- schedule -------------------------------------------------
        q8r_0, k8r_0 = alloc_qk()
        q8r_1, k8r_1 = alloc_qk()
        o0 = alloc_o()
        o1 = alloc_o()
        vts0, vts1 = [], []
        # batch-0 mc0 q/k (startup critical), then first v tiles
        for s in proj_qk_stages(0, 0, q8r_0[0], k8r_0[0]):
            s()
        proj_v(0, [0, 1, 2], vts0)
        qk01 = proj_qk_stages(0, 1, q8r_0[1], k8r_0[1])
        qk10 = proj_qk_stages(1, 0, q8r_1[0], k8r_1[0])
        qk11 = proj_qk_stages(1, 1, q8r_1[1], k8r_1[1])
        run_head(attn_head(0, 0, q8r_0, k8r_0, vts0, o0),
                 [lambda: proj_v(0, [3, 4], vts0),
                  lambda: proj_v(0, [5, 6], vts0),
                  lambda: proj_v(0, [7], vts0),
                  qk01[0], qk01[1], qk01[2], qk01[3]])
        run_head(attn_head(0, 1, q8r_0, k8r_0, vts0, o0),
                 [qk10[0], qk10[1], qk10[2], qk10[3]])
        run_head(attn_head(0, 2, q8r_0, k8r_0, vts0, o0),
                 [lambda: proj_v(1, [0, 1], vts1),
                  lambda: proj_v(1, [2, 3], vts1),
                  qk11[0], qk11[1], qk11[2], qk11[3]])
        run_head(attn_head(0, 3, q8r_0, k8r_0, vts0, o0),
                 [lambda: proj_v(1, [4, 5], vts1),
                  lambda: proj_v(1, [6, 7], vts1)])
        # batch 1
        run_head(attn_head(1, 0, q8r_1, k8r_1, vts1, o1),
                 [lambda: zproj(0, o0, (0,)),
                  lambda: zproj(0, o0, (1,))])
        run_head(attn_head(1, 1, q8r_1, k8r_1, vts1, o1))
        run_head(attn_head(1, 2, q8r_1, k8r_1, vts1, o1))
        run_head(attn_head(1, 3, q8r_1, k8r_1, vts1, o1))
        zproj(1, o1)

    nc.finalize()
    return nc


def _get_nc():
    if "nc" not in _CACHE:
        _CACHE["nc"] = _build_nc()
    return _CACHE["nc"]


def kernel(x, y, w_qkv, w_out, b_out):
    from concourse.bass_utils import run_bass_kernel_spmd

    nc = _get_nc()

    x = np.asarray(x, dtype=np.float32).reshape(16, C, N).astype(np.float16)
    y = np.asarray(y, dtype=np.float32).reshape(16, C, N).astype(np.float16)
    w_qkv = np.asarray(w_qkv, dtype=np.float32)
    wq_t = np.ascontiguousarray(w_qkv[0:HID].T).astype(np.float16)
    wk_t = np.ascontiguousarray(w_qkv[HID:2 * HID].T).astype(np.float16)
    wv_t = np.ascontiguousarray(w_qkv[2 * HID:3 * HID].T).astype(np.float16)
    wo_t = np.ascontiguousarray(np.asarray(w_out, dtype=np.float32).T).astype(np.float16)
    bo = np.ascontiguousarray(
        np.asarray(b_out, dtype=np.float32).reshape(2, 128, 1))

    in_maps = []
    for c in range(NCORES):
        in_maps.append({
            "x": np.ascontiguousarray(x[c * NB:(c + 1) * NB]),
            "y": np.ascontiguousarray(y[c * NB:(c + 1) * NB]),
            "wq_t": wq_t, "wk_t": wk_t, "wv_t": wv_t, "wo_t": wo_t,
            "b_out": bo,
        })

    res = run_bass_kernel_spmd(nc, in_maps, list(range(NCORES)))
    full = np.concatenate([res.results[i]["out"] for i in range(NCORES)], axis=0)
    return full.reshape(16, C, 32, 32)


# revision 14
# speedup vs baseline: 1.0655x; 1.0089x over previous
"""Cross-attention kernel for 8 trn2 NeuronCores.

Reference computation (per batch b of 16):
  q = Wq @ x, k = Wk @ y, v = Wv @ y          (1x1 convs as channel matmuls)
  q,k l2-normalized over the SPATIAL axis (per (h,d) row)
  sim = 10 * q^T k per head; attn = softmax_j(sim); o = attn @ v^T
  out = Wo @ o + b

Sharding: data-parallel over batch, 2 batches per core, weights replicated.

v2 design (ACT-bound plan; TimelineSim cost model):
  - exp runs on ACT only: 64 tiles x [128,1024] = 66.4us/core busy floor.
    Everything else is shaped to hide under it.
  - S_T (q^T k, contraction d=64) in fp8e4m3 with MatmulPerfMode.DoubleRow
    (0.5 cycles/row): q,k are separately l2-normalized (per hd-row scale via
    bn_stats + Quake rsqrt), quantized PSUM->SBUF fp8 by one tensor_scalar,
    then DMA-reshuffled (via DRAM scratch) into the DoubleRow pair layout
    [32p, 2pair, n] per head (d = p + 32*pair).
  - softmax denominator: ones-column in v^T (row 64 of PV out), normalized by
    gpsimd partition_broadcast of the PSUM denominator row + one DVE
    tensor_tensor divide per head (no DMA round trips).
  - PSUM (16KB/partition): st [128,1024]f32 bufs=2 (8K) + acc [128,512]f32
    bufs=2 (4K, proj/zproj halves) + op [65,1024]f32 bufs=1 (4K, PV accum).
    S_T writes 2KB-aligned [64,512] blocks (own start/stop groups).
  - PE emission inside a head is one-ahead: S_T(jc+1) is emitted before
    PV(jc) so ACT streams exps back to back.
  - PSUM->SBUF raw q/k copies and v^T copies run on gpsimd (Pool) to keep
    DVE under the ACT roofline.
"""

import sys

import numpy as np

if "/opt/trn_rl_repo" not in sys.path:
    sys.path.insert(0, "/opt/trn_rl_repo")

NB = 2        # batches per core
C = 256       # channels
N = 1024      # spatial (32*32)
HEADS = 4
DH = 64
HID = 256
NCORES = 8
MAGIC = 0x5F3759DF  # Quake fast inverse-sqrt seed

_CACHE = {}


def _quake_rsqrt(nc, pool, p_ap, out_ap, final_scale):
    """out = rsqrt(p) * final_scale for [128,1] fp32 APs, DVE-only.

    Quake seed + 2 Newton iterations (rel err ~1e-7), no ACT table needed.
    """
    from concourse import mybir

    i32 = mybir.dt.int32
    alu = mybir.AluOpType
    t = pool.tile([128, 1], mybir.dt.float32, tag="qk_rs_t", bufs=4)
    r = pool.tile([128, 1], mybir.dt.float32, tag="qk_rs_r", bufs=4)
    a = pool.tile([128, 1], mybir.dt.float32, tag="qk_rs_a", bufs=4)
    # seed: r0 = bitcast(MAGIC - (bitcast_i32(p) >> 1))
    nc.vector.tensor_scalar(t.bitcast(i32), p_ap.bitcast(i32), 1, None,
                            alu.logical_shift_right)
    nc.vector.tensor_scalar(r.bitcast(i32), t.bitcast(i32), -1, MAGIC,
                            alu.mult, alu.add)
    # Newton 1: r = r * (1.5 - 0.5 * p * r^2)
    nc.vector.scalar_tensor_tensor(a[:], r[:], r[:, 0:1], p_ap,
                                   alu.mult, alu.mult)
    nc.vector.tensor_scalar(a[:], a[:], -0.5, 1.5, alu.mult, alu.add)
    nc.vector.tensor_scalar(t[:], a[:], r[:, 0:1], None, alu.mult)
    # Newton 2 (fold final_scale into the last multiply)
    nc.vector.scalar_tensor_tensor(a[:], t[:], t[:, 0:1], p_ap,
                                   alu.mult, alu.mult)
    nc.vector.tensor_scalar(a[:], a[:], -0.5, 1.5, alu.mult, alu.add)
    nc.vector.tensor_scalar(out_ap, a[:], t[:, 0:1], final_scale,
                            alu.mult, alu.mult)


def _build_nc():
    from contextlib import ExitStack

    import concourse.tile as tile
    from concourse import bacc, mybir

    f32 = mybir.dt.float32
    f16 = mybir.dt.float16
    f8 = mybir.dt.float8e4
    alu = mybir.AluOpType
    EXP = mybir.ActivationFunctionType.Exp
    DR = mybir.MatmulPerfMode.DoubleRow

    nc = bacc.Bacc("TRN2", target_bir_lowering=False)

    xin = nc.dram_tensor("x", [NB, C, N], f16, kind="ExternalInput")
    yin = nc.dram_tensor("y", [NB, C, N], f16, kind="ExternalInput")
    wq = nc.dram_tensor("wq_t", [C, HID], f16, kind="ExternalInput")
    wk = nc.dram_tensor("wk_t", [C, HID], f16, kind="ExternalInput")
    wv = nc.dram_tensor("wv_t", [C, HID], f16, kind="ExternalInput")
    wo = nc.dram_tensor("wo_t", [HID, C], f16, kind="ExternalInput")
    bo = nc.dram_tensor("b_out", [2, 128, 1], f32, kind="ExternalInput")
    out = nc.dram_tensor("out", [NB, C, N], f32, kind="ExternalOutput")
    # DRAM scratch for the fp8 DoubleRow pair-layout reshuffle
    q8d = nc.dram_tensor("q8_scratch", [NB, 2, 128, N], f8, kind="Internal")
    k8d = nc.dram_tensor("k8_scratch", [NB, 2, 128, N], f8, kind="Internal")

    with tile.TileContext(nc) as tc, ExitStack() as ctx:
        consts = ctx.enter_context(tc.tile_pool(name="consts", bufs=1))
        big = ctx.enter_context(tc.tile_pool(name="big", bufs=2))
        sm = ctx.enter_context(tc.tile_pool(name="sm", bufs=4))
        ps = ctx.enter_context(tc.tile_pool(name="ps", bufs=2, space="PSUM"))

        # ---- weight + input loads (k/q path first: critical) ----------
        wq_sb = consts.tile([128, 2, HID], f16, tag="wq")
        wk_sb = consts.tile([128, 2, HID], f16, tag="wk")
        wv_sb = consts.tile([128, 2, HID], f16, tag="wv")
        wo_sb = consts.tile([128, 2, C], f16, tag="wo")
        b_sb = consts.tile([128, 2, 1], f32, tag="bo")
        # warm the ACT exp table while input DMAs are in flight
        warm = sm.tile([128, 1], f32, tag="warm", bufs=1)
        nc.vector.memset(warm[:], 0.0)
        nc.scalar.activation(out=warm[:], in_=warm[:], func=EXP, scale=1.0)
        xts, yts = [], []
        for nb in range(NB):
            xt = big.tile([128, 2, N], f16, tag="xt", bufs=2)
            yt = big.tile([128, 2, N], f16, tag="yt", bufs=2)
            xts.append(xt)
            yts.append(yt)
        nc.sync.dma_start(out=wq_sb[:], in_=wq.rearrange("(kc p) n -> p kc n", p=128))
        for kc in range(2):
            nc.sync.dma_start(out=xts[0][:, kc, :], in_=xin[0, kc * 128:(kc + 1) * 128, :])
        nc.sync.dma_start(out=wk_sb[:], in_=wk.rearrange("(kc p) n -> p kc n", p=128))
        for kc in range(2):
            nc.sync.dma_start(out=yts[0][:, kc, :], in_=yin[0, kc * 128:(kc + 1) * 128, :])
        nc.sync.dma_start(out=wv_sb[:], in_=wv.rearrange("(kc p) n -> p kc n", p=128))
        nc.sync.dma_start(out=wo_sb[:], in_=wo.rearrange("(kc p) n -> p kc n", p=128))
        nc.sync.dma_start(out=b_sb[:], in_=bo.rearrange("kc p n -> p kc n"))
        nc.sync.dma_start(out=yts[1][:], in_=yin[1].rearrange("(kc p) n -> p kc n", p=128))
        nc.sync.dma_start(out=xts[1][:], in_=xin[1].rearrange("(kc p) n -> p kc n", p=128))

        # ---------------------------------------------------------------
        # proj_qk: q,k channel matmuls for one 128-row hd chunk (2 heads),
        # separate l2 scales, fp8 quantize, DoubleRow reshuffle via DRAM.
        # q8r/k8r: [64, 2, N] fp8; partitions [ha*32,(ha+1)*32) = head
        # 2*mc+ha, pair dim = d split (d = p + 32*pair).
        # Emitted in 4 stages so PSUM 'acc' ring waits never head-block the
        # PE stream of a concurrently-running attention head.
        def _proj_one(nb, mc, w_sb, src, raw_tag, t8_tag, dram, t8r, state):
            def mm():
                state["ps"] = []
                for ih in range(2):
                    pp = ps.tile([128, 512], f32, tag="acc", bufs=2, name="pp")
                    for kc in range(2):
                        nc.tensor.matmul(
                            pp[:],
                            w_sb[:, kc, mc * 128:(mc + 1) * 128],
                            src[:, kc, ih * 512:(ih + 1) * 512],
                            start=(kc == 0), stop=(kc == 1))
                    state["ps"].append(pp)

            def quant():
                pps = state["ps"]
                st = sm.tile([128, 2, 6], f32, tag="st" + raw_tag, bufs=4, name="st" + raw_tag)
                for ih in range(2):
                    nc.vector.bn_stats(out=st[:, ih, :], in_=pps[ih][:])
                mv = sm.tile([128, 2], f32, tag="mv" + raw_tag, bufs=4, name="mv" + raw_tag)
                nc.vector.bn_aggr(out=mv[:], in_=st[:])
                # sumsq/N = mean^2 + var; scale = rsqrt(u)/sqrt(N)
                u = sm.tile([128, 1], f32, tag="u" + raw_tag, bufs=4, name="u" + raw_tag)
                nc.vector.scalar_tensor_tensor(u[:], mv[:, 0:1], mv[:, 0:1],
                                               mv[:, 1:2], alu.mult, alu.add)
                sc = sm.tile([128, 1], f32, tag="s" + raw_tag, bufs=4, name="s" + raw_tag)
                _quake_rsqrt(nc, sm, u[:], sc[:], 1.0 / 32.0)
                t8 = big.tile([128, N], f8, tag=t8_tag, bufs=2, name=t8_tag)
                for ih in range(2):
                    nc.vector.tensor_scalar(t8[:, ih * 512:(ih + 1) * 512],
                                            pps[ih][:], sc[:, 0:1], None, alu.mult)
                nc.sync.dma_start(out=dram[nb, mc], in_=t8[:])
                for ha in range(2):
                    nc.sync.dma_start(
                        out=t8r[ha * 32:(ha + 1) * 32, :, :],
                        in_=dram[nb, mc, ha * 64:(ha + 1) * 64, :].rearrange(
                            "(i p) n -> p i n", i=2, p=32))

            return mm, quant

        def proj_qk_stages(nb, mc, q8r, k8r):
            stq, stk = {}, {}
            qmm, qquant = _proj_one(nb, mc, wq_sb, xts[nb], "qraw", "q8",
                                    q8d, q8r, stq)
            kmm, kquant = _proj_one(nb, mc, wk_sb, yts[nb], "kraw", "k8",
                                    k8d, k8r, stk)
            return [qmm, qquant, kmm, kquant]

        def proj_v(nb, jcs, vts_nb):
            for jc in jcs:
                vp = ps.tile([128, 512], f32, tag="acc", bufs=2)
                for kc in range(2):
                    nc.tensor.matmul(
                        vp[:, 0:HID],
                        yts[nb][:, kc, jc * 128:(jc + 1) * 128],
                        wv_sb[:, kc, :],
                        start=(kc == 0), stop=(kc == 1))
                vt = big.tile([128, 4, 128], f16, tag="vt", bufs=16)
                nc.vector.tensor_copy(vt[:, :, 0:64],
                                      vp[:, 0:HID].rearrange("p (h d) -> p h d", h=4))
                nc.gpsimd.memset(vt[:, :, 64:128], 1.0)
                vts_nb.append(vt)

        # one attention head, emitted as a generator of "steps" so PE filler
        # work can be interleaved between jc iterations without delaying the
        # ACT-critical S_T / exp chain.
        def attn_head(nb, h, q8r2, k8r2, vts_nb, o_sb):
            mc, ha = h // 2, h % 2
            q8r, k8r = q8r2[mc], k8r2[mc]
            hp, hr = mc, 64 * ha

            def st_mm(jc):
                st = ps.tile([128, N], f32, tag="st", bufs=2)
                for ih in range(2):
                    nc.tensor.matmul(
                        st[:, ih * 512:(ih + 1) * 512],
                        k8r[ha * 32:(ha + 1) * 32, :, jc * 128:(jc + 1) * 128],
                        q8r[ha * 32:(ha + 1) * 32, :, ih * 512:(ih + 1) * 512],
                        start=True, stop=True, perf_mode=DR)
                return st

            op = ps.tile([128, N], f32, tag="op", bufs=1)
            sts = [st_mm(0)]
            for jc in range(8):
                et = big.tile([128, N], f16, tag="et", bufs=6)
                nc.scalar.activation(out=et[:], in_=sts[jc][:], func=EXP, scale=10.0)
                if jc < 7:
                    sts.append(st_mm(jc + 1))
                vt = vts_nb[jc]
                for ih in range(2):
                    nc.tensor.matmul(
                        op[:, ih * 512:(ih + 1) * 512],
                        vt[:, h, :],
                        et[:, ih * 512:(ih + 1) * 512],
                        start=(jc == 0), stop=(jc == 7))
                yield jc
            # softmax normalize: denominator is pre-broadcast on rows 64:128
            # (64-wide ones block in vt); reciprocal into SBUF, then multiply
            db = big.tile([64, N], f32, tag="db", bufs=2, name="db")
            nc.vector.reciprocal(db[:], op[64:128, :])
            nc.vector.tensor_tensor(o_sb[hr:hr + 64, hp, :], op[0:64, :],
                                    db[:], alu.mult)
            yield -1

        def run_head(gen, fillers=()):
            """Drive a head generator, emitting one filler callable per step."""
            fi = iter(fillers)
            for _ in gen:
                f = next(fi, None)
                if f is not None:
                    f()
            for f in fi:
                f()

        def zproj(nb, o_sb, ihs=(0, 1)):
            for mc in range(2):
                for ih in ihs:
                    zp = ps.tile([128, 512], f32, tag="acc", bufs=2)
                    for kc in range(2):
                        nc.tensor.matmul(
                            zp[:],
                            wo_sb[:, kc, mc * 128:(mc + 1) * 128],
                            o_sb[:, kc, ih * 512:(ih + 1) * 512],
                            start=(kc == 0), stop=(kc == 1))
                    zs = big.tile([128, 512], f32, tag="zs", bufs=4)
                    nc.vector.tensor_scalar(zs[:], zp[:], b_sb[:, mc, 0:1], None,
                                            alu.add)
                    nc.sync.dma_start(
                        out=out[nb, mc * 128:(mc + 1) * 128, ih * 512:(ih + 1) * 512],
                        in_=zs[:])

        def alloc_qk():
            q8r2 = [big.tile([64, 2, N], f8, tag="q8r", bufs=4, name=f"q8r{i}")
                    for i in range(2)]
            k8r2 = [big.tile([64, 2, N], f8, tag="k8r", bufs=4, name=f"k8r{i}")
                    for i in range(2)]
            return q8r2, k8r2

        def alloc_o():
            return big.tile([128, 2, N], f16, tag="osb", bufs=2, name="osb")

        # ---- schedule -------------------------------------------------
        q8r_0, k8r_0 = alloc_qk()
        q8r_1, k8r_1 = alloc_qk()
        o0 = alloc_o()
        o1 = alloc_o()
        vts0, vts1 = [], []
        # batch-0 mc0 q/k (startup critical), then first v tiles
        for s in proj_qk_stages(0, 0, q8r_0[0], k8r_0[0]):
            s()
        proj_v(0, [0, 1, 2], vts0)
        qk01 = proj_qk_stages(0, 1, q8r_0[1], k8r_0[1])
        qk10 = proj_qk_stages(1, 0, q8r_1[0], k8r_1[0])
        qk11 = proj_qk_stages(1, 1, q8r_1[1], k8r_1[1])
        run_head(attn_head(0, 0, q8r_0, k8r_0, vts0, o0),
                 [lambda: proj_v(0, [3, 4], vts0),
                  lambda: proj_v(0, [5, 6], vts0),
                  lambda: proj_v(0, [7], vts0),
                  qk01[0], qk01[1], qk01[2], qk01[3]])
        run_head(attn_head(0, 1, q8r_0, k8r_0, vts0, o0),
                 [qk10[0], qk10[1], qk10[2], qk10[3]])
        run_head(attn_head(0, 2, q8r_0, k8r_0, vts0, o0),
                 [lambda: proj_v(1, [0, 1], vts1),
                  lambda: proj_v(1, [2, 3], vts1),
                  qk11[0], qk11[1], qk11[2], qk11[3]])
        run_head(attn_head(0, 3, q8r_0, k8r_0, vts0, o0),
                 [lambda: proj_v(1, [4, 5], vts1),
                  lambda: proj_v(1, [6, 7], vts1)])
        # batch 1
        run_head(attn_head(1, 0, q8r_1, k8r_1, vts1, o1),
                 [lambda: zproj(0, o0, (0,)),
                  lambda: zproj(0, o0, (1,))])
        run_head(attn_head(1, 1, q8r_1, k8r_1, vts1, o1))
        run_head(attn_head(1, 2, q8r_1, k8r_1, vts1, o1))
        run_head(attn_head(1, 3, q8r_1, k8r_1, vts1, o1))
        zproj(1, o1)

    nc.finalize()
    return nc


def _get_nc():
    if "nc" not in _CACHE:
        _CACHE["nc"] = _build_nc()
    return _CACHE["nc"]


def kernel(x, y, w_qkv, w_out, b_out):
    from concourse.bass_utils import run_bass_kernel_spmd

    nc = _get_nc()

    x = np.asarray(x, dtype=np.float32).reshape(16, C, N).astype(np.float16)
    y = np.asarray(y, dtype=np.float32).reshape(16, C, N).astype(np.float16)
    w_qkv = np.asarray(w_qkv, dtype=np.float32)
    wq_t = np.ascontiguousarray(w_qkv[0:HID].T).astype(np.float16)
    wk_t = np.ascontiguousarray(w_qkv[HID:2 * HID].T).astype(np.float16)
    wv_t = np.ascontiguousarray(w_qkv[2 * HID:3 * HID].T).astype(np.float16)
    wo_t = np.ascontiguousarray(np.asarray(w_out, dtype=np.float32).T).astype(np.float16)
    bo = np.ascontiguousarray(
        np.asarray(b_out, dtype=np.float32).reshape(2, 128, 1))

    in_maps = []
    for c in range(NCORES):
        in_maps.append({
            "x": np.ascontiguousarray(x[c * NB:(c + 1) * NB]),
            "y": np.ascontiguousarray(y[c * NB:(c + 1) * NB]),
            "wq_t": wq_t, "wk_t": wk_t, "wv_t": wv_t, "wo_t": wo_t,
            "b_out": bo,
        })

    res = run_bass_kernel_spmd(nc, in_maps, list(range(NCORES)))
    full = np.concatenate([res.results[i]["out"] for i in range(NCORES)], axis=0)
    return full.reshape(16, C, 32, 32)


# revision 15
# speedup vs baseline: 1.0657x; 1.0003x over previous
"""Cross-attention kernel for 8 trn2 NeuronCores.

Reference computation (per batch b of 16):
  q = Wq @ x, k = Wk @ y, v = Wv @ y          (1x1 convs as channel matmuls)
  q,k l2-normalized over the SPATIAL axis (per (h,d) row)
  sim = 10 * q^T k per head; attn = softmax_j(sim); o = attn @ v^T
  out = Wo @ o + b

Sharding: data-parallel over batch, 2 batches per core, weights replicated.

v3 design (ACT-bound; optimized against the TimelineSim cost model):
  - exp on ACT is the roofline: 64 x [128,1024] tiles = 66.4us busy/core.
  - S_T (q^T k, d=64 contraction) in fp8e4m3 + MatmulPerfMode.DoubleRow
    (0.5 cycles/row, dst partition base must be 0): q quantized RAW (q~N(0,1)
    fits e4m3), k carries the combined l2 scale sq*sk*2^10 (power-of-two gain
    keeps k8 in range; exp scale becomes 10/1024). DoubleRow pair layout
    [32p, 2pair, n] per head (d = p + 32*pair) via a DRAM-scratch reshuffle.
  - softmax denominator: 64-wide ones block in v^T puts the denominator
    pre-broadcast on PV out rows 64:128; normalize = reciprocal + multiply
    per i-half (DVE, PSUM-sourced).
  - zproj contracts kc1 as two 64-row matmuls (heads 2 and 3 separately) so
    the drain tail only waits on the last head's 64-row matmul.
  - Cross-head S_T handoff: head h emits head h+1's first S_T before its own
    last PV, so ACT never gaps at head boundaries (st ring bufs=2 fits this).
  - PSUM (16KB/partition): st [128,1024]f32 x2 (8K) + acc [128,512]f32 x2
    (4K) + op [128,1024]f32 x1 (4K).
  - Weights packed into 2 DMAs (wqk, wvo); batch-1 x/y loads deferred into
    head (0,0) so the fp8 reshuffle round-trip owns the DMA engine early.
"""

import sys

import numpy as np

if "/opt/trn_rl_repo" not in sys.path:
    sys.path.insert(0, "/opt/trn_rl_repo")

NB = 2        # batches per core
C = 256       # channels
N = 1024      # spatial (32*32)
HEADS = 4
DH = 64
HID = 256
NCORES = 8
MAGIC = 0x5F3759DF  # Quake fast inverse-sqrt seed
KGAIN = 1024.0      # power-of-two gain folded into k8; exp scale = 10/KGAIN

_CACHE = {}


def _quake_rsqrt(nc, pool, p_ap, out_ap, final_scale):
    """out = rsqrt(p) * final_scale for [128,1] fp32 APs, DVE-only.

    Quake seed + 2 Newton iterations (rel err ~1e-7), no ACT table needed.
    """
    from concourse import mybir

    i32 = mybir.dt.int32
    alu = mybir.AluOpType
    t = pool.tile([128, 1], mybir.dt.float32, tag="qk_rs_t", bufs=4)
    r = pool.tile([128, 1], mybir.dt.float32, tag="qk_rs_r", bufs=4)
    a = pool.tile([128, 1], mybir.dt.float32, tag="qk_rs_a", bufs=4)
    # seed: r0 = bitcast(MAGIC - (bitcast_i32(p) >> 1))
    nc.vector.tensor_scalar(t.bitcast(i32), p_ap.bitcast(i32), 1, None,
                            alu.logical_shift_right)
    nc.vector.tensor_scalar(r.bitcast(i32), t.bitcast(i32), -1, MAGIC,
                            alu.mult, alu.add)
    # Newton 1: r = r * (1.5 - 0.5 * p * r^2)
    nc.vector.scalar_tensor_tensor(a[:], r[:], r[:, 0:1], p_ap,
                                   alu.mult, alu.mult)
    nc.vector.tensor_scalar(a[:], a[:], -0.5, 1.5, alu.mult, alu.add)
    nc.vector.tensor_scalar(t[:], a[:], r[:, 0:1], None, alu.mult)
    # Newton 2 (fold final_scale into the last multiply)
    nc.vector.scalar_tensor_tensor(a[:], t[:], t[:, 0:1], p_ap,
                                   alu.mult, alu.mult)
    nc.vector.tensor_scalar(a[:], a[:], -0.5, 1.5, alu.mult, alu.add)
    nc.vector.tensor_scalar(out_ap, a[:], t[:, 0:1], final_scale,
                            alu.mult, alu.mult)


def _build_nc():
    from contextlib import ExitStack

    import concourse.tile as tile
    from concourse import bacc, mybir

    f32 = mybir.dt.float32
    f16 = mybir.dt.float16
    f8 = mybir.dt.float8e4
    alu = mybir.AluOpType
    EXP = mybir.ActivationFunctionType.Exp
    DR = mybir.MatmulPerfMode.DoubleRow

    nc = bacc.Bacc("TRN2", target_bir_lowering=False)

    xin = nc.dram_tensor("x", [NB, C, N], f16, kind="ExternalInput")
    yin = nc.dram_tensor("y", [NB, C, N], f16, kind="ExternalInput")
    wqk = nc.dram_tensor("wqk", [128, 4, HID], f16, kind="ExternalInput")
    wvo = nc.dram_tensor("wvo", [128, 4, HID], f16, kind="ExternalInput")
    bo = nc.dram_tensor("b_out", [2, 128, 1], f32, kind="ExternalInput")
    out = nc.dram_tensor("out", [NB, C, N], f32, kind="ExternalOutput")
    # DRAM scratch for the fp8 DoubleRow pair-layout reshuffle
    q8d = nc.dram_tensor("q8_scratch", [NB, 2, 128, N], f8, kind="Internal")
    k8d = nc.dram_tensor("k8_scratch", [NB, 2, 128, N], f8, kind="Internal")

    with tile.TileContext(nc) as tc, ExitStack() as ctx:
        consts = ctx.enter_context(tc.tile_pool(name="consts", bufs=1))
        big = ctx.enter_context(tc.tile_pool(name="big", bufs=2))
        sm = ctx.enter_context(tc.tile_pool(name="sm", bufs=4))
        ps = ctx.enter_context(tc.tile_pool(name="ps", bufs=2, space="PSUM"))

        # ---- weight + input loads ------------------------------------
        wqk_sb = consts.tile([128, 4, HID], f16, tag="wqk")
        wvo_sb = consts.tile([128, 4, HID], f16, tag="wvo")
        b_sb = consts.tile([128, 2, 1], f32, tag="bo")
        # warm the ACT exp table while input DMAs are in flight
        warm = sm.tile([128, 1], f32, tag="warm", bufs=1)
        nc.vector.memset(warm[:], 0.0)
        nc.scalar.activation(out=warm[:], in_=warm[:], func=EXP, scale=1.0)
        xts, yts = [], []
        for nb in range(NB):
            xt = big.tile([128, 2, N], f16, tag="xt", bufs=2)
            yt = big.tile([128, 2, N], f16, tag="yt", bufs=2)
            xts.append(xt)
            yts.append(yt)
        nc.sync.dma_start(out=wqk_sb[:], in_=wqk[:])
        nc.sync.dma_start(out=xts[0][:], in_=xin[0].rearrange("(kc p) n -> p kc n", p=128))
        nc.sync.dma_start(out=yts[0][:], in_=yin[0].rearrange("(kc p) n -> p kc n", p=128))
        nc.sync.dma_start(out=wvo_sb[:], in_=wvo[:])
        nc.sync.dma_start(out=b_sb[:], in_=bo.rearrange("kc p n -> p kc n"))

        def load_b1():
            nc.sync.dma_start(out=yts[1][:], in_=yin[1].rearrange("(kc p) n -> p kc n", p=128))
            nc.sync.dma_start(out=xts[1][:], in_=xin[1].rearrange("(kc p) n -> p kc n", p=128))

        # ---------------------------------------------------------------
        # q/k projection for one 128-row hd chunk (heads 2mc, 2mc+1).
        # q8 = raw fp8 copy of the projection; k8 carries sq*sk*KGAIN.
        # Both are DMA-reshuffled (via DRAM) into DoubleRow pair layout
        # [64, 2, N]: partition 32a+p, pair i  <->  hd row 64a + 32i + p.
        # Emitted in 4 stages so PSUM 'acc' ring waits never head-block
        # the PE stream of a concurrently-running attention head.
        def proj_qk_stages(nb, mc, q8r, k8r):
            state = {}

            def reshuffle(t8, dram):
                nc.sync.dma_start(out=dram[nb, mc], in_=t8[:])
                for ha in range(2):
                    nc.sync.dma_start(
                        out=(q8r if dram is q8d else k8r)[ha * 32:(ha + 1) * 32, :, :],
                        in_=dram[nb, mc, ha * 64:(ha + 1) * 64, :].rearrange(
                            "(i p) n -> p i n", i=2, p=32))

            def pmm(w4, srct, key):
                pps = []
                for ih in range(2):
                    pp = ps.tile([128, 512], f32, tag="acc", bufs=2, name="pp")
                    for kc in range(2):
                        nc.tensor.matmul(
                            pp[:],
                            wqk_sb[:, w4 + kc, mc * 128:(mc + 1) * 128],
                            srct[:, kc, ih * 512:(ih + 1) * 512],
                            start=(kc == 0), stop=(kc == 1))
                    pps.append(pp)
                state[key] = pps

            def qmm():
                pmm(0, xts[nb], "q")

            def qquant():
                qps = state["q"]
                q8 = big.tile([128, N], f8, tag="q8", bufs=2, name="q8")
                for ih in range(2):
                    nc.vector.tensor_copy(q8[:, ih * 512:(ih + 1) * 512],
                                          qps[ih][:])
                reshuffle(q8, q8d)
                stq = sm.tile([128, 2, 6], f32, tag="stq", bufs=4, name="stq")
                for ih in range(2):
                    nc.vector.bn_stats(out=stq[:, ih, :], in_=qps[ih][:])
                mvq = sm.tile([128, 2], f32, tag="mvq", bufs=4, name="mvq")
                nc.vector.bn_aggr(out=mvq[:], in_=stq[:])
                uq = sm.tile([128, 1], f32, tag="uq", bufs=4, name="uq")
                nc.vector.scalar_tensor_tensor(uq[:], mvq[:, 0:1], mvq[:, 0:1],
                                               mvq[:, 1:2], alu.mult, alu.add)
                state["uq"] = uq

            def kmm():
                pmm(2, yts[nb], "k")

            def kquant():
                kps = state["k"]
                stk = sm.tile([128, 2, 6], f32, tag="stk", bufs=4, name="stk")
                for ih in range(2):
                    nc.vector.bn_stats(out=stk[:, ih, :], in_=kps[ih][:])
                mvk = sm.tile([128, 2], f32, tag="mvk", bufs=4, name="mvk")
                nc.vector.bn_aggr(out=mvk[:], in_=stk[:])
                # pqk = (sumsq_q/N) * (sumsq_k/N); combined scale
                # sq*sk*KGAIN = rsqrt(pqk) * KGAIN/N
                pqk = sm.tile([128, 1], f32, tag="pqk", bufs=4, name="pqk")
                nc.vector.scalar_tensor_tensor(pqk[:], mvk[:, 0:1], mvk[:, 0:1],
                                               mvk[:, 1:2], alu.mult, alu.add)
                nc.vector.tensor_tensor(pqk[:], pqk[:], state["uq"][:], alu.mult)
                sck = sm.tile([128, 1], f32, tag="sck", bufs=4, name="sck")
                _quake_rsqrt(nc, sm, pqk[:], sck[:], KGAIN / float(N))
                k8 = big.tile([128, N], f8, tag="k8", bufs=2, name="k8")
                for ih in range(2):
                    nc.vector.tensor_scalar(k8[:, ih * 512:(ih + 1) * 512],
                                            kps[ih][:], sck[:, 0:1], None,
                                            alu.mult)
                reshuffle(k8, k8d)

            return [qmm, qquant, kmm, kquant]

        def proj_v(nb, jcs, vts_nb):
            for jc in jcs:
                vp = ps.tile([128, 512], f32, tag="acc", bufs=2)
                for kc in range(2):
                    nc.tensor.matmul(
                        vp[:, 0:HID],
                        yts[nb][:, kc, jc * 128:(jc + 1) * 128],
                        wvo_sb[:, kc, :],
                        start=(kc == 0), stop=(kc == 1))
                vt = big.tile([128, 4, 128], f16, tag="vt", bufs=16)
                nc.vector.tensor_copy(vt[:, :, 0:64],
                                      vp[:, 0:HID].rearrange("p (h d) -> p h d", h=4))
                nc.gpsimd.memset(vt[:, :, 64:128], 1.0)
                vts_nb.append(vt)

        # one attention head; generator of steps so PE filler work can be
        # interleaved between jc iterations. Head h+1's first S_T is emitted
        # by head h (before its last PV) via the hctx handoff.
        def make_head(nb, h, q8r2, k8r2, vts_nb, o_sb):
            mc, ha = h // 2, h % 2
            q8r, k8r = q8r2[mc], k8r2[mc]

            def st_mm(jc):
                st = ps.tile([128, N], f32, tag="st", bufs=2)
                for ih in range(2):
                    nc.tensor.matmul(
                        st[:, ih * 512:(ih + 1) * 512],
                        k8r[ha * 32:(ha + 1) * 32, :, jc * 128:(jc + 1) * 128],
                        q8r[ha * 32:(ha + 1) * 32, :, ih * 512:(ih + 1) * 512],
                        start=True, stop=True, perf_mode=DR)
                return st

            return {"nb": nb, "h": h, "mc": mc, "ha": ha, "vts": vts_nb,
                    "o_sb": o_sb, "st_mm": st_mm, "first_st": None}

        def head_gen(hc, next_hc):
            h, ha, hp = hc["h"], hc["ha"], hc["mc"]
            hr = 64 * ha
            o_sb, vts_nb = hc["o_sb"], hc["vts"]
            op = ps.tile([128, N], f32, tag="op", bufs=1)
            sts = [hc["first_st"] if hc["first_st"] is not None
                   else hc["st_mm"](0)]
            for jc in range(8):
                et = big.tile([128, N], f16, tag="et", bufs=6)
                nc.scalar.activation(out=et[:], in_=sts[jc][:], func=EXP,
                                     scale=10.0 / KGAIN)
                if jc < 7:
                    sts.append(hc["st_mm"](jc + 1))
                elif next_hc is not None:
                    next_hc["first_st"] = next_hc["st_mm"](0)
                vt = vts_nb[jc]
                for ih in range(2):
                    nc.tensor.matmul(
                        op[:, ih * 512:(ih + 1) * 512],
                        vt[:, h, :],
                        et[:, ih * 512:(ih + 1) * 512],
                        start=(jc == 0), stop=(jc == 7))
                yield jc
            # softmax normalize per i-half: denominator is pre-broadcast on
            # rows 64:128 (64-wide ones block in vt)
            for ihn in range(2):
                sl = slice(ihn * 512, (ihn + 1) * 512)
                db = big.tile([64, 512], f32, tag="db", bufs=4, name="db")
                nc.vector.reciprocal(db[:], op[64:128, sl])
                nc.vector.tensor_tensor(o_sb[hr:hr + 64, hp, sl],
                                        op[0:64, sl], db[:], alu.mult)
            yield -1

        def run_head(gen, fillers=()):
            fi = iter(fillers)
            for _ in gen:
                f = next(fi, None)
                if f is not None:
                    f()
            for f in fi:
                f()

        def zproj(nb, o_sb, ihs=(0, 1)):
            # kc0 contracts heads 0,1; kc1 is split into heads 2 and 3 so the
            # final drain only waits on the last head's 64-row matmul.
            for mc in range(2):
                for ih in ihs:
                    sl = slice(ih * 512, (ih + 1) * 512)
                    msl = slice(mc * 128, (mc + 1) * 128)
                    zp = ps.tile([128, 512], f32, tag="acc", bufs=2)
                    nc.tensor.matmul(zp[:], wvo_sb[:, 2, msl],
                                     o_sb[:, 0, sl], start=True, stop=False)
                    nc.tensor.matmul(zp[:], wvo_sb[0:64, 3, msl],
                                     o_sb[0:64, 1, sl], start=False, stop=False)
                    nc.tensor.matmul(zp[:], wvo_sb[64:128, 3, msl],
                                     o_sb[64:128, 1, sl], start=False, stop=True)
                    zs = big.tile([128, 512], f32, tag="zs", bufs=4)
                    nc.vector.tensor_scalar(zs[:], zp[:], b_sb[:, mc, 0:1],
                                            None, alu.add)
                    nc.sync.dma_start(
                        out=out[nb, msl, sl],
                        in_=zs[:])

        def alloc_qk():
            q8r2 = [big.tile([64, 2, N], f8, tag="q8r", bufs=4, name=f"q8r{i}")
                    for i in range(2)]
            k8r2 = [big.tile([64, 2, N], f8, tag="k8r", bufs=4, name=f"k8r{i}")
                    for i in range(2)]
            return q8r2, k8r2

        def alloc_o():
            return big.tile([128, 2, N], f16, tag="osb", bufs=2, name="osb")

        # ---- schedule -------------------------------------------------
        q8r_0, k8r_0 = alloc_qk()
        q8r_1, k8r_1 = alloc_qk()
        o0 = alloc_o()
        o1 = alloc_o()
        vts0, vts1 = [], []
        # batch-0 mc0 q/k (startup critical), then first v tiles
        for s in proj_qk_stages(0, 0, q8r_0[0], k8r_0[0]):
            s()
        proj_v(0, [0, 1, 2], vts0)
        qk01 = proj_qk_stages(0, 1, q8r_0[1], k8r_0[1])
        qk10 = proj_qk_stages(1, 0, q8r_1[0], k8r_1[0])
        qk11 = proj_qk_stages(1, 1, q8r_1[1], k8r_1[1])
        hcs = [make_head(0, hh, q8r_0, k8r_0, vts0, o0) for hh in range(4)]
        hcs += [make_head(1, hh, q8r_1, k8r_1, vts1, o1) for hh in range(4)]
        fillers = [
            [load_b1,
             lambda: proj_v(0, [3, 4], vts0),
             lambda: proj_v(0, [5, 6], vts0),
             lambda: proj_v(0, [7], vts0),
             qk01[0], qk01[1], qk01[2], qk01[3]],
            [qk10[0], qk10[1], qk10[2], qk10[3]],
            [lambda: proj_v(1, [0, 1], vts1),
             lambda: proj_v(1, [2, 3], vts1),
             qk11[0], qk11[1], qk11[2], qk11[3]],
            [lambda: proj_v(1, [4, 5], vts1),
             lambda: proj_v(1, [6, 7], vts1)],
            [lambda: zproj(0, o0, (0,)),
             lambda: zproj(0, o0, (1,))],
            [], [], [],
        ]
        for i, hc in enumerate(hcs):
            nxt = hcs[i + 1] if i + 1 < len(hcs) else None
            run_head(head_gen(hc, nxt), fillers[i])
        zproj(1, o1)

    nc.finalize()
    return nc


def _get_nc():
    if "nc" not in _CACHE:
        _CACHE["nc"] = _build_nc()
    return _CACHE["nc"]


def kernel(x, y, w_qkv, w_out, b_out):
    from concourse.bass_utils import run_bass_kernel_spmd

    nc = _get_nc()

    x = np.asarray(x, dtype=np.float32).reshape(16, C, N).astype(np.float16)
    y = np.asarray(y, dtype=np.float32).reshape(16, C, N).astype(np.float16)
    w_qkv = np.asarray(w_qkv, dtype=np.float32)
    wq_t = np.ascontiguousarray(w_qkv[0:HID].T).astype(np.float16)
    wk_t = np.ascontiguousarray(w_qkv[HID:2 * HID].T).astype(np.float16)
    wv_t = np.ascontiguousarray(w_qkv[2 * HID:3 * HID].T).astype(np.float16)
    wo_t = np.ascontiguousarray(np.asarray(w_out, dtype=np.float32).T).astype(np.float16)
    bo = np.ascontiguousarray(
        np.asarray(b_out, dtype=np.float32).reshape(2, 128, 1))

    def pack2(a, b):
        # [128, 4, 256]: [:, 0:2] = a chunks, [:, 2:4] = b chunks, where
        # [:, w*2+kc, n] = w_t[kc*128 + p, n]
        pk = np.empty((128, 4, HID), dtype=np.float16)
        pk[:, 0:2] = a.reshape(2, 128, HID).transpose(1, 0, 2)
        pk[:, 2:4] = b.reshape(2, 128, HID).transpose(1, 0, 2)
        return pk

    wqk = pack2(wq_t, wk_t)
    wvo = pack2(wv_t, wo_t)

    in_maps = []
    for c in range(NCORES):
        in_maps.append({
            "x": np.ascontiguousarray(x[c * NB:(c + 1) * NB]),
            "y": np.ascontiguousarray(y[c * NB:(c + 1) * NB]),
            "wqk": wqk, "wvo": wvo,
            "b_out": bo,
        })

    res = run_bass_kernel_spmd(nc, in_maps, list(range(NCORES)))
    full = np.concatenate([res.results[i]["out"] for i in range(NCORES)], axis=0)
    return full.reshape(16, C, 32, 32)


# revision 16
# speedup vs baseline: 1.0685x; 1.0026x over previous
"""Cross-attention kernel for 8 trn2 NeuronCores.

Reference computation (per batch b of 16):
  q = Wq @ x, k = Wk @ y, v = Wv @ y          (1x1 convs as channel matmuls)
  q,k l2-normalized over the SPATIAL axis (per (h,d) row)
  sim = 10 * q^T k per head; attn = softmax_j(sim); o = attn @ v^T
  out = Wo @ o + b

Sharding: data-parallel over batch, 2 batches per core, weights replicated.

v3 design (ACT-bound; optimized against the TimelineSim cost model):
  - exp on ACT is the roofline: 64 x [128,1024] tiles = 66.4us busy/core.
  - S_T (q^T k, d=64 contraction) in fp8e4m3 + MatmulPerfMode.DoubleRow
    (0.5 cycles/row, dst partition base must be 0): q quantized RAW (q~N(0,1)
    fits e4m3), k carries the combined l2 scale sq*sk*2^10 (power-of-two gain
    keeps k8 in range; exp scale becomes 10/1024). DoubleRow pair layout
    [32p, 2pair, n] per head (d = p + 32*pair) via a DRAM-scratch reshuffle.
  - softmax denominator: 64-wide ones block in v^T puts the denominator
    pre-broadcast on PV out rows 64:128; normalize = reciprocal + multiply
    per i-half (DVE, PSUM-sourced).
  - zproj contracts kc1 as two 64-row matmuls (heads 2 and 3 separately) so
    the drain tail only waits on the last head's 64-row matmul.
  - Cross-head S_T handoff: head h emits head h+1's first S_T before its own
    last PV, so ACT never gaps at head boundaries (st ring bufs=2 fits this).
  - PSUM (16KB/partition): st [128,1024]f32 x2 (8K) + acc [128,512]f32 x2
    (4K) + op [128,1024]f32 x1 (4K).
  - Weights packed into 2 DMAs (wqk, wvo); batch-1 x/y loads deferred into
    head (0,0) so the fp8 reshuffle round-trip owns the DMA engine early.
"""

import sys

import numpy as np

if "/opt/trn_rl_repo" not in sys.path:
    sys.path.insert(0, "/opt/trn_rl_repo")

NB = 2        # batches per core
C = 256       # channels
N = 1024      # spatial (32*32)
HEADS = 4
DH = 64
HID = 256
NCORES = 8
MAGIC = 0x5F3759DF  # Quake fast inverse-sqrt seed
KGAIN = 1024.0      # power-of-two gain folded into k8; exp scale = 10/KGAIN

_CACHE = {}


def _quake_rsqrt(nc, pool, p_ap, out_ap, final_scale):
    """out = rsqrt(p) * final_scale for [128,1] fp32 APs, DVE-only.

    Quake seed + 2 Newton iterations (rel err ~1e-7), no ACT table needed.
    """
    from concourse import mybir

    i32 = mybir.dt.int32
    alu = mybir.AluOpType
    t = pool.tile([128, 1], mybir.dt.float32, tag="qk_rs_t", bufs=4)
    r = pool.tile([128, 1], mybir.dt.float32, tag="qk_rs_r", bufs=4)
    a = pool.tile([128, 1], mybir.dt.float32, tag="qk_rs_a", bufs=4)
    # seed: r0 = bitcast(MAGIC - (bitcast_i32(p) >> 1))
    nc.vector.tensor_scalar(t.bitcast(i32), p_ap.bitcast(i32), 1, None,
                            alu.logical_shift_right)
    nc.vector.tensor_scalar(r.bitcast(i32), t.bitcast(i32), -1, MAGIC,
                            alu.mult, alu.add)
    # Newton 1: r = r * (1.5 - 0.5 * p * r^2)
    nc.vector.scalar_tensor_tensor(a[:], r[:], r[:, 0:1], p_ap,
                                   alu.mult, alu.mult)
    nc.vector.tensor_scalar(a[:], a[:], -0.5, 1.5, alu.mult, alu.add)
    nc.vector.tensor_scalar(t[:], a[:], r[:, 0:1], None, alu.mult)
    # Newton 2 (fold final_scale into the last multiply)
    nc.vector.scalar_tensor_tensor(a[:], t[:], t[:, 0:1], p_ap,
                                   alu.mult, alu.mult)
    nc.vector.tensor_scalar(a[:], a[:], -0.5, 1.5, alu.mult, alu.add)
    nc.vector.tensor_scalar(out_ap, a[:], t[:, 0:1], final_scale,
                            alu.mult, alu.mult)


def _build_nc():
    from contextlib import ExitStack

    import concourse.tile as tile
    from concourse import bacc, mybir

    f32 = mybir.dt.float32
    f16 = mybir.dt.float16
    f8 = mybir.dt.float8e4
    alu = mybir.AluOpType
    EXP = mybir.ActivationFunctionType.Exp
    DR = mybir.MatmulPerfMode.DoubleRow

    nc = bacc.Bacc("TRN2", target_bir_lowering=False)

    xin = nc.dram_tensor("x", [NB, C, N], f16, kind="ExternalInput")
    yin = nc.dram_tensor("y", [NB, C, N], f16, kind="ExternalInput")
    wqk = nc.dram_tensor("wqk", [128, 4, HID], f16, kind="ExternalInput")
    wvo = nc.dram_tensor("wvo", [128, 4, HID], f16, kind="ExternalInput")
    bo = nc.dram_tensor("b_out", [2, 128, 1], f32, kind="ExternalInput")
    out = nc.dram_tensor("out", [NB, C, N], f32, kind="ExternalOutput")
    # DRAM scratch for the fp8 DoubleRow pair-layout reshuffle
    q8d = nc.dram_tensor("q8_scratch", [NB, 2, 128, N], f8, kind="Internal")
    k8d = nc.dram_tensor("k8_scratch", [NB, 2, 128, N], f8, kind="Internal")

    with tile.TileContext(nc) as tc, ExitStack() as ctx:
        consts = ctx.enter_context(tc.tile_pool(name="consts", bufs=1))
        big = ctx.enter_context(tc.tile_pool(name="big", bufs=2))
        sm = ctx.enter_context(tc.tile_pool(name="sm", bufs=4))
        ps = ctx.enter_context(tc.tile_pool(name="ps", bufs=2, space="PSUM"))

        # ---- weight + input loads ------------------------------------
        wqk_sb = consts.tile([128, 4, HID], f16, tag="wqk")
        wvo_sb = consts.tile([128, 4, HID], f16, tag="wvo")
        b_sb = consts.tile([128, 2, 1], f32, tag="bo")
        # warm the ACT exp table while input DMAs are in flight
        warm = sm.tile([128, 1], f32, tag="warm", bufs=1)
        nc.vector.memset(warm[:], 0.0)
        nc.scalar.activation(out=warm[:], in_=warm[:], func=EXP, scale=1.0)
        xts, yts = [], []
        for nb in range(NB):
            xt = big.tile([128, 2, N], f16, tag="xt", bufs=2)
            yt = big.tile([128, 2, N], f16, tag="yt", bufs=2)
            xts.append(xt)
            yts.append(yt)
        nc.sync.dma_start(out=wqk_sb[:], in_=wqk[:])
        nc.sync.dma_start(out=xts[0][:], in_=xin[0].rearrange("(kc p) n -> p kc n", p=128))
        nc.sync.dma_start(out=yts[0][:], in_=yin[0].rearrange("(kc p) n -> p kc n", p=128))
        nc.sync.dma_start(out=wvo_sb[:], in_=wvo[:])
        nc.sync.dma_start(out=b_sb[:], in_=bo.rearrange("kc p n -> p kc n"))

        def load_b1():
            nc.sync.dma_start(out=yts[1][:], in_=yin[1].rearrange("(kc p) n -> p kc n", p=128))
            nc.sync.dma_start(out=xts[1][:], in_=xin[1].rearrange("(kc p) n -> p kc n", p=128))

        # ---------------------------------------------------------------
        # q/k projection for one 128-row hd chunk (heads 2mc, 2mc+1).
        # q8 = raw fp8 copy of the projection; k8 carries sq*sk*KGAIN.
        # Both are DMA-reshuffled (via DRAM) into DoubleRow pair layout
        # [64, 2, N]: partition 32a+p, pair i  <->  hd row 64a + 32i + p.
        # Emitted in 4 stages so PSUM 'acc' ring waits never head-block
        # the PE stream of a concurrently-running attention head.
        def proj_qk_stages(nb, mc, q8r, k8r):
            state = {}

            def reshuffle(t8, dram):
                nc.sync.dma_start(out=dram[nb, mc], in_=t8[:])
                for ha in range(2):
                    nc.sync.dma_start(
                        out=(q8r if dram is q8d else k8r)[ha * 32:(ha + 1) * 32, :, :],
                        in_=dram[nb, mc, ha * 64:(ha + 1) * 64, :].rearrange(
                            "(i p) n -> p i n", i=2, p=32))

            def pmm(w4, srct, key):
                pps = []
                for ih in range(2):
                    pp = ps.tile([128, 512], f32, tag="acc", bufs=2, name="pp")
                    for kc in range(2):
                        nc.tensor.matmul(
                            pp[:],
                            wqk_sb[:, w4 + kc, mc * 128:(mc + 1) * 128],
                            srct[:, kc, ih * 512:(ih + 1) * 512],
                            start=(kc == 0), stop=(kc == 1))
                    pps.append(pp)
                state[key] = pps

            def qmm():
                pmm(0, xts[nb], "q")

            def qquant():
                qps = state["q"]
                q8 = big.tile([128, N], f8, tag="q8", bufs=2, name="q8")
                for ih in range(2):
                    nc.vector.tensor_copy(q8[:, ih * 512:(ih + 1) * 512],
                                          qps[ih][:])
                reshuffle(q8, q8d)
                stq = sm.tile([128, 2, 6], f32, tag="stq", bufs=4, name="stq")
                for ih in range(2):
                    nc.vector.bn_stats(out=stq[:, ih, :], in_=qps[ih][:])
                mvq = sm.tile([128, 2], f32, tag="mvq", bufs=4, name="mvq")
                nc.vector.bn_aggr(out=mvq[:], in_=stq[:])
                uq = sm.tile([128, 1], f32, tag="uq", bufs=4, name="uq")
                nc.vector.scalar_tensor_tensor(uq[:], mvq[:, 0:1], mvq[:, 0:1],
                                               mvq[:, 1:2], alu.mult, alu.add)
                state["uq"] = uq

            def kmm():
                pmm(2, yts[nb], "k")

            def kquant():
                kps = state["k"]
                stk = sm.tile([128, 2, 6], f32, tag="stk", bufs=4, name="stk")
                for ih in range(2):
                    nc.vector.bn_stats(out=stk[:, ih, :], in_=kps[ih][:])
                mvk = sm.tile([128, 2], f32, tag="mvk", bufs=4, name="mvk")
                nc.vector.bn_aggr(out=mvk[:], in_=stk[:])
                # pqk = (sumsq_q/N) * (sumsq_k/N); combined scale
                # sq*sk*KGAIN = rsqrt(pqk) * KGAIN/N
                pqk = sm.tile([128, 1], f32, tag="pqk", bufs=4, name="pqk")
                nc.vector.scalar_tensor_tensor(pqk[:], mvk[:, 0:1], mvk[:, 0:1],
                                               mvk[:, 1:2], alu.mult, alu.add)
                nc.vector.tensor_tensor(pqk[:], pqk[:], state["uq"][:], alu.mult)
                sck = sm.tile([128, 1], f32, tag="sck", bufs=4, name="sck")
                _quake_rsqrt(nc, sm, pqk[:], sck[:], KGAIN / float(N))
                k8 = big.tile([128, N], f8, tag="k8", bufs=2, name="k8")
                for ih in range(2):
                    nc.vector.tensor_scalar(k8[:, ih * 512:(ih + 1) * 512],
                                            kps[ih][:], sck[:, 0:1], None,
                                            alu.mult)
                reshuffle(k8, k8d)

            return [qmm, qquant, kmm, kquant]

        def proj_v(nb, jcs, vts_nb):
            for jc in jcs:
                vp = ps.tile([128, 512], f32, tag="acc", bufs=2)
                for kc in range(2):
                    nc.tensor.matmul(
                        vp[:, 0:HID],
                        yts[nb][:, kc, jc * 128:(jc + 1) * 128],
                        wvo_sb[:, kc, :],
                        start=(kc == 0), stop=(kc == 1))
                vt = big.tile([128, 4, 128], f16, tag="vt", bufs=16)
                nc.vector.tensor_copy(vt[:, :, 0:64],
                                      vp[:, 0:HID].rearrange("p (h d) -> p h d", h=4))
                nc.gpsimd.memset(vt[:, :, 64:128], 1.0)
                vts_nb.append(vt)

        # one attention head; generator of steps so PE filler work can be
        # interleaved between jc iterations. Head h+1's first S_T is emitted
        # by head h (before its last PV) via the hctx handoff.
        def make_head(nb, h, q8r2, k8r2, vts_nb, o_sb):
            mc, ha = h // 2, h % 2
            q8r, k8r = q8r2[mc], k8r2[mc]

            def st_mm(jc):
                st = ps.tile([128, N], f32, tag="st", bufs=2)
                for ih in range(2):
                    nc.tensor.matmul(
                        st[:, ih * 512:(ih + 1) * 512],
                        k8r[ha * 32:(ha + 1) * 32, :, jc * 128:(jc + 1) * 128],
                        q8r[ha * 32:(ha + 1) * 32, :, ih * 512:(ih + 1) * 512],
                        start=True, stop=True, perf_mode=DR)
                return st

            return {"nb": nb, "h": h, "mc": mc, "ha": ha, "vts": vts_nb,
                    "o_sb": o_sb, "st_mm": st_mm, "first_st": None}

        def head_gen(hc, next_hc):
            h, ha, hp = hc["h"], hc["ha"], hc["mc"]
            hr = 64 * ha
            o_sb, vts_nb = hc["o_sb"], hc["vts"]
            op = ps.tile([128, N], f32, tag="op", bufs=1)
            sts = [hc["first_st"] if hc["first_st"] is not None
                   else hc["st_mm"](0)]
            for jc in range(8):
                et = big.tile([128, N], f16, tag="et", bufs=6)
                nc.scalar.activation(out=et[:], in_=sts[jc][:], func=EXP,
                                     scale=10.0 / KGAIN)
                if jc < 7:
                    sts.append(hc["st_mm"](jc + 1))
                elif next_hc is not None:
                    next_hc["first_st"] = next_hc["st_mm"](0)
                vt = vts_nb[jc]
                for ih in range(2):
                    nc.tensor.matmul(
                        op[:, ih * 512:(ih + 1) * 512],
                        vt[:, h, :],
                        et[:, ih * 512:(ih + 1) * 512],
                        start=(jc == 0), stop=(jc == 7))
                yield jc
            # softmax normalize per i-half: denominator is pre-broadcast on
            # rows 64:128 (64-wide ones block in vt)
            for ihn in range(2):
                sl = slice(ihn * 512, (ihn + 1) * 512)
                db = big.tile([64, 512], f32, tag="db", bufs=4, name="db")
                nc.vector.reciprocal(db[:], op[64:128, sl])
                nc.vector.tensor_tensor(o_sb[hr:hr + 64, hp, sl],
                                        op[0:64, sl], db[:], alu.mult)
            yield -1

        def run_head(gen, fillers=()):
            fi = iter(fillers)
            for _ in gen:
                f = next(fi, None)
                if f is not None:
                    f()
            for f in fi:
                f()

        def zproj(nb, o_sb, ihs=(0, 1)):
            # kc0 contracts heads 0,1; kc1 is split into heads 2 and 3 so the
            # final drain only waits on the last head's 64-row matmul.
            for mc in range(2):
                for ih in ihs:
                    sl = slice(ih * 512, (ih + 1) * 512)
                    msl = slice(mc * 128, (mc + 1) * 128)
                    zp = ps.tile([128, 512], f32, tag="acc", bufs=2)
                    nc.tensor.matmul(zp[:], wvo_sb[:, 2, msl],
                                     o_sb[:, 0, sl], start=True, stop=False)
                    nc.tensor.matmul(zp[:], wvo_sb[:, 3, msl],
                                     o_sb[:, 1, sl], start=False, stop=True)
                    zs = big.tile([128, 512], f32, tag="zs", bufs=4)
                    nc.vector.tensor_scalar(zs[:], zp[:], b_sb[:, mc, 0:1],
                                            None, alu.add)
                    nc.sync.dma_start(
                        out=out[nb, msl, sl],
                        in_=zs[:])

        def alloc_qk():
            q8r2 = [big.tile([64, 2, N], f8, tag="q8r", bufs=4, name=f"q8r{i}")
                    for i in range(2)]
            k8r2 = [big.tile([64, 2, N], f8, tag="k8r", bufs=4, name=f"k8r{i}")
                    for i in range(2)]
            return q8r2, k8r2

        def alloc_o():
            return big.tile([128, 2, N], f16, tag="osb", bufs=2, name="osb")

        # ---- schedule -------------------------------------------------
        q8r_0, k8r_0 = alloc_qk()
        q8r_1, k8r_1 = alloc_qk()
        o0 = alloc_o()
        o1 = alloc_o()
        vts0, vts1 = [], []
        # batch-0 mc0 q/k (startup critical), then first v tiles
        for s in proj_qk_stages(0, 0, q8r_0[0], k8r_0[0]):
            s()
        proj_v(0, [0, 1, 2], vts0)
        qk01 = proj_qk_stages(0, 1, q8r_0[1], k8r_0[1])
        qk10 = proj_qk_stages(1, 0, q8r_1[0], k8r_1[0])
        qk11 = proj_qk_stages(1, 1, q8r_1[1], k8r_1[1])
        hcs = [make_head(0, hh, q8r_0, k8r_0, vts0, o0) for hh in range(4)]
        hcs += [make_head(1, hh, q8r_1, k8r_1, vts1, o1) for hh in range(4)]
        fillers = [
            [load_b1,
             lambda: proj_v(0, [3, 4], vts0),
             lambda: proj_v(0, [5, 6], vts0),
             lambda: proj_v(0, [7], vts0),
             qk01[0], qk01[1], qk01[2], qk01[3]],
            [qk10[0], qk10[1], qk10[2], qk10[3]],
            [lambda: proj_v(1, [0, 1], vts1),
             lambda: proj_v(1, [2, 3], vts1),
             qk11[0], qk11[1], qk11[2], qk11[3]],
            [lambda: proj_v(1, [4, 5], vts1),
             lambda: proj_v(1, [6, 7], vts1)],
            [lambda: zproj(0, o0, (0,)),
             lambda: zproj(0, o0, (1,))],
            [], [], [],
        ]
        for i, hc in enumerate(hcs):
            nxt = hcs[i + 1] if i + 1 < len(hcs) else None
            run_head(head_gen(hc, nxt), fillers[i])
        zproj(1, o1)

    nc.finalize()
    return nc


def _get_nc():
    if "nc" not in _CACHE:
        _CACHE["nc"] = _build_nc()
    return _CACHE["nc"]


def kernel(x, y, w_qkv, w_out, b_out):
    from concourse.bass_utils import run_bass_kernel_spmd

    nc = _get_nc()

    x = np.asarray(x, dtype=np.float32).reshape(16, C, N).astype(np.float16)
    y = np.asarray(y, dtype=np.float32).reshape(16, C, N).astype(np.float16)
    w_qkv = np.asarray(w_qkv, dtype=np.float32)
    wq_t = np.ascontiguousarray(w_qkv[0:HID].T).astype(np.float16)
    wk_t = np.ascontiguousarray(w_qkv[HID:2 * HID].T).astype(np.float16)
    wv_t = np.ascontiguousarray(w_qkv[2 * HID:3 * HID].T).astype(np.float16)
    wo_t = np.ascontiguousarray(np.asarray(w_out, dtype=np.float32).T).astype(np.float16)
    bo = np.ascontiguousarray(
        np.asarray(b_out, dtype=np.float32).reshape(2, 128, 1))

    def pack2(a, b):
        # [128, 4, 256]: [:, 0:2] = a chunks, [:, 2:4] = b chunks, where
        # [:, w*2+kc, n] = w_t[kc*128 + p, n]
        pk = np.empty((128, 4, HID), dtype=np.float16)
        pk[:, 0:2] = a.reshape(2, 128, HID).transpose(1, 0, 2)
        pk[:, 2:4] = b.reshape(2, 128, HID).transpose(1, 0, 2)
        return pk

    wqk = pack2(wq_t, wk_t)
    wvo = pack2(wv_t, wo_t)

    in_maps = []
    for c in range(NCORES):
        in_maps.append({
            "x": np.ascontiguousarray(x[c * NB:(c + 1) * NB]),
            "y": np.ascontiguousarray(y[c * NB:(c + 1) * NB]),
            "wqk": wqk, "wvo": wvo,
            "b_out": bo,
        })

    res = run_bass_kernel_spmd(nc, in_maps, list(range(NCORES)))
    full = np.concatenate([res.results[i]["out"] for i in range(NCORES)], axis=0)
    return full.reshape(16, C, 32, 32)
